# revision 1
# baseline (speedup 1.0000x reference)
"""Trainium2 Bass kernel for nn_KmerEmbed: conv1d(one-hot kmer filters) + relu + window-sum.

Computes, for seqs (32,32,30,21), weight (8000,20,3), bias (8000,):
  out[n,m,f] = sum_l relu( sum_{a,j} seqs[n,m,l+j,a(<20)]*weight[f,a,j] + bias[f] )
with l over the 28 valid conv positions; returns (32,32,8000) float32.

Strategy (8 NeuronCores, data-parallel over the 1024 flattened rows, 128 rows/core):
  - im2col on host: X[(j,a)+bias_row, tile, (n4,l28)] -> stationary operand of a
    K=61 matmul streamed against the replicated filter matrix Wt (61,8000) in
    float32r (1 cycle/row at N>=256, ~1e-4 rel precision).
  - conv tiles are packed in pairs into PE row-groups (partitions 0-60 / 64-124)
    so two matmuls stream concurrently.
  - relu(conv) evaluated from PSUM by ScalarE (activation Relu) and VectorE
    (tensor_scalar max) in parallel, written to SBUF as float16.
  - window-sum via a second matmul with 0/1 selection matrices G (112,32) in
    fp16, one PE column-group per 32-row output block; 8 tiles accumulate into
    each column group of a single (128, chunk) PSUM tile, so the final
    PSUM->SBUF copy covers all 128 partitions at once.
  - staging (128,8000) in SBUF, contiguous DMA to DRAM.
"""

import os
import sys

import numpy as np
from numpy.lib.stride_tricks import sliding_window_view

for _p in ("/opt/trn_rl_repo", "/root/.axon_site/_ro/trn_rl_repo"):
    if os.path.isdir(_p) and _p not in sys.path:
        sys.path.insert(0, _p)

import concourse.bacc as bacc
import concourse.mybir as mybir
from concourse.tile import TileContext
from concourse.bass_utils import run_bass_kernel_spmd

# problem sizes (hardcoded per spec)
N_, M_, L_, B_ = 32, 32, 30, 21
A_, K_ = 20, 3
F_ = 8000
NM = N_ * M_              # 1024
CORES = 8
NMC = NM // CORES         # 128 rows per core
LOUT = L_ - K_ + 1        # 28
NMG = 4                   # rows per conv tile
MT = NMG * LOUT           # 112 psum partitions per conv tile
NT = NMC // NMG           # 32 tiles per core
KC = A_ * K_ + 1          # 61 = 60 + bias row
FCH = 500                 # matmul free-dim chunk (one PSUM bank)
BIG = 1000                # relu/copy chunk (two banks)
NBIG = F_ // BIG          # 8

_f32r = mybir.dt.float32r
_f32 = mybir.dt.float32
_f16 = mybir.dt.float16

_cached_nc = None


def _build_program():
    nc = bacc.Bacc("TRN2", target_bir_lowering=False, debug=False,
                   num_devices=CORES)
    xin_d = nc.declare_dram_parameter("xin", [128, NT // 2 * MT], _f16,
                                      isOutput=False)
    wt_d = nc.declare_dram_parameter("wt", [128, F_], _f16, isOutput=False)
    g_d = nc.declare_dram_parameter("g", [MT, 8 * 32], _f16, isOutput=False)
    out_d = nc.declare_dram_parameter("out", [NMC, F_], _f32, isOutput=True)

    relu_fn = mybir.ActivationFunctionType.Relu
    max_op = mybir.AluOpType.max

    with TileContext(nc) as tc:
        with tc.tile_pool(name="const", bufs=1) as cpool, \
             tc.tile_pool(name="rbuf", bufs=18) as rpool, \
             tc.tile_pool(name="stage", bufs=1) as spool, \
             tc.tile_pool(name="pconv", bufs=3, space="PSUM") as pconv, \
             tc.tile_pool(name="psum", bufs=1, space="PSUM") as psump:
            xin_sb = cpool.tile([128, NT // 2 * MT], _f16)
            wt_sb = cpool.tile([128, F_], _f16)
            g_sb = cpool.tile([MT, 8 * 32], _f16)
            stage = spool.tile([NMC, F_], _f32)
            nc.sync.dma_start(out=xin_sb[:], in_=xin_d[:])
            nc.sync.dma_start(out=g_sb[:], in_=g_d[:])
            for i in range(4):
                s = slice(i * (F_ // 4), (i + 1) * (F_ // 4))
                nc.sync.dma_start(out=wt_sb[:, s], in_=wt_d[:, s])

            sum_order = [g0 * 8 + i for i in range(8) for g0 in range(4)]
            first_in_group = set(range(0, NT, 8))

            # pair visit order rotates across the 4 column groups so the
            # window-sum matmuls (which accumulate per column group) can chase
            # the relus instead of bursting at the chunk boundary.
            pair_order = [g + r for r in range(4) for g in [0, 4, 8, 12]]
            for c in range(NBIG):
                rtiles = {}
                ps = psump.tile([128, 1024], _f32)
                started = set()
                n_summed = 0
                for p in pair_order:
                    pc_e = pconv.tile([MT, 1024], _f32, tag="pc")
                    pc_o = pconv.tile([MT, 1024], _f32, tag="pc")
                    for h in range(2):
                        fs = slice(c * BIG + h * FCH, c * BIG + (h + 1) * FCH)
                        hs = slice(h * 512, h * 512 + FCH)
                        nc.tensor.matmul(
                            out=pc_e[:, hs],
                            lhsT=xin_sb[0:KC, p * MT:(p + 1) * MT],
                            rhs=wt_sb[0:KC, fs], start=True, stop=True)
                        nc.tensor.matmul(
                            out=pc_o[:, hs],
                            lhsT=xin_sb[64:64 + KC, p * MT:(p + 1) * MT],
                            rhs=wt_sb[64:64 + KC, fs], start=True, stop=True)
                    r_e = rpool.tile([MT, 1024], _f16, tag="re")
                    r_o = rpool.tile([MT, 1024], _f16, tag="ro")
                    nc.scalar.activation(out=r_e[:, 0:1012], in_=pc_e[:, 0:1012],
                                         func=relu_fn)
                    o_eng = nc.scalar if p == pair_order[-1] else nc.vector
                    if o_eng is nc.scalar:
                        nc.scalar.activation(out=r_o[:, 0:1012],
                                             in_=pc_o[:, 0:1012], func=relu_fn)
                    else:
                        nc.vector.tensor_scalar(out=r_o[:, 0:1012],
                                                in0=pc_o[:, 0:1012],
                                                scalar1=0.0, scalar2=None,
                                                op0=max_op)
                    rtiles[2 * p] = r_e
                    rtiles[2 * p + 1] = r_o
                    # once a full round of 4 pairs (one per column group) is
                    # done, emit their 8 window-sum matmuls (4-way concurrent)
                    if len(rtiles) % 8 == 0:
                        round_pairs = pair_order[len(rtiles) // 2 - 4:
                                                 len(rtiles) // 2]
                        for h in range(2):
                            hs = slice(h * 512, h * 512 + FCH)
                            for dt_ in range(2):
                                for rp in round_pairs:
                                    t = 2 * rp + dt_
                                    grp = t // 8
                                    oi = t % 8
                                    n_summed += 1
                                    is_first = (grp, h) not in started
                                    started.add((grp, h))
                                    nc.tensor.matmul(
                                        out=ps[32 * grp:32 * grp + 32, hs],
                                        lhsT=g_sb[:, 32 * oi:32 * oi + 32],
                                        rhs=rtiles[t][:, hs],
                                        start=is_first,
                                        stop=(n_summed == 2 * NT),
                                        skip_group_check=True,
                                        tile_position=(0, 32 * grp))
                for h in range(2):
                    eng = nc.vector
                    eng.tensor_copy(
                        out=stage[:, c * BIG + h * FCH:c * BIG + (h + 1) * FCH],
                        in_=ps[:, h * 512:h * 512 + FCH])
                if c % 2 == 1:
                    osl = slice((c - 1) * BIG, (c + 1) * BIG)
                    nc.sync.dma_start(out=out_d[:, osl], in_=stage[:, osl])

    nc.compile()
    return nc


def _get_program():
    global _cached_nc
    if _cached_nc is None:
        _cached_nc = _build_program()
    return _cached_nc


def _host_prep(seqs, weight, bias):
    s = np.asarray(seqs, np.float32).reshape(NM, L_, B_)[:, :, :A_]
    sw = sliding_window_view(s, K_, axis=1)          # (NM, 28, 20, 3)
    X = sw.transpose(3, 2, 0, 1).reshape(A_ * K_, NM, LOUT)
    X = np.concatenate([X, np.ones((1, NM, LOUT), np.float32)], axis=0)

    Wt = np.asarray(weight, np.float32).transpose(2, 1, 0).reshape(A_ * K_, F_)
    Wb = np.concatenate([Wt, np.asarray(bias, np.float32)[None, :]], axis=0)
    wt = np.zeros((128, F_), np.float32)
    wt[0:KC] = Wb
    wt[64:64 + KC] = Wb
    wt_f16 = wt.astype(np.float16)

    G = np.zeros((MT, 8 * 32), np.float16)
    for oi in range(8):
        for n in range(NMG):
            G[n * LOUT:(n + 1) * LOUT, 32 * oi + 4 * oi + n] = 1.0

    in_maps = []
    for c in range(CORES):
        Xc = X[:, c * NMC:(c + 1) * NMC, :].reshape(KC, NT, MT)
        xin = np.zeros((128, NT // 2, MT), np.float32)
        xin[0:KC] = Xc[:, 0::2]
        xin[64:64 + KC] = Xc[:, 1::2]
        in_maps.append({
            "xin": np.ascontiguousarray(xin.reshape(128, NT // 2 * MT)).astype(np.float16),
            "wt": wt_f16,
            "g": G,
        })
    return in_maps


def run_bass(seqs, weight, bias, trace=False):
    """Returns (out (32,32,8000) float32, exec_time_ns or None)."""
    nc = _get_program()
    in_maps = _host_prep(seqs, weight, bias)
    res = run_bass_kernel_spmd(nc, in_maps, list(range(CORES)), trace=trace)
    out = np.concatenate([res.results[c]["out"] for c in range(CORES)], axis=0)
    return out.reshape(N_, M_, F_), res.exec_time_ns


def kernel(seqs, weight, bias):
    out, _ = run_bass(seqs, weight, bias, trace=False)
    return out



# revision 2
# speedup vs baseline: 1.1751x; 1.1751x over previous
"""Trainium2 Bass kernel for nn_KmerEmbed: conv1d(one-hot kmer filters) + relu + window-sum.

Computes, for seqs (32,32,30,21), weight (8000,20,3), bias (8000,):
  out[n,m,f] = sum_l relu( sum_{a,j} seqs[n,m,l+j,a(<20)]*weight[f,a,j] + bias[f] )
with l over the 28 valid conv positions; returns (32,32,8000) float32.

Strategy (8 NeuronCores, data-parallel over the 1024 flattened rows, 128 rows/core):
  - im2col on host: the 128 rows x 28 positions = 3584 (nm,l) pairs are packed
    densely into 28 tiles of 128 PSUM partitions (full PE width), K=61 rows
    (60 one-hot taps + bias row).
  - conv = matmul vs the replicated filter matrix Wb (61,8000) f16; tiles are
    packed in pairs into PE row-groups (partitions 0-60 / 64-124) so two
    matmuls stream concurrently; f chunked by 500 (one PSUM bank per matmul),
    e/o halves of a 2-bank pair tile.
  - relu(conv) from PSUM by ScalarE (activation Relu) and VectorE (tensor_scalar
    max) in parallel, written to SBUF as float16, one instr per pair (FD=1012).
  - window-sum via matmul with 0/1 selection matrices G (128,32) f16, K=128:
    tile t feeds output column-group t//7 (4 col-groups run concurrently);
    7 tiles accumulate per group into one (128,500) PSUM bank per chunk.
    Pairs are visited in an order that rotates across the 4 col groups.
  - PSUM->SBUF copy converts to f16; DMA f16 to DRAM; host casts to f32.
"""

import os
import sys

import numpy as np
from numpy.lib.stride_tricks import sliding_window_view

for _p in ("/opt/trn_rl_repo", "/root/.axon_site/_ro/trn_rl_repo"):
    if os.path.isdir(_p) and _p not in sys.path:
        sys.path.insert(0, _p)

import concourse.bacc as bacc
import concourse.mybir as mybir
from concourse.tile import TileContext
from concourse.bass_utils import run_bass_kernel_spmd

# problem sizes (hardcoded per spec)
N_, M_, L_, B_ = 32, 32, 30, 21
A_, K_ = 20, 3
F_ = 8000
NM = N_ * M_              # 1024
CORES = 8
NMC = NM // CORES         # 128 rows per core
LOUT = L_ - K_ + 1        # 28
FLAT = NMC * LOUT         # 3584 (nm,l) positions per core
NT = FLAT // 128          # 28 tiles of 128 positions
NP = NT // 2              # 14 e/o pairs
TPG = NT // 4             # 7 tiles accumulate per output col group
KC = A_ * K_ + 1          # 61 = 60 + bias row
FCH = 500                 # matmul free-dim chunk (one PSUM bank)
NCH = F_ // FCH           # 16 chunks

_f32 = mybir.dt.float32
_f16 = mybir.dt.float16

# relu engine per rotated-pair position (6 on vector, 8 on scalar)
DVE_POS = frozenset((0, 3, 5, 7, 9, 12))

# pair visit order rotating across the 4 sum col-groups; straddling pairs last
PAIR_ORDER = [0, 4, 7, 11, 1, 5, 8, 12, 2, 6, 9, 13, 3, 10]

_cached_nc = None


def _build_program():
    nc = bacc.Bacc("TRN2", target_bir_lowering=False, debug=False,
                   num_devices=CORES)
    xin_d = nc.declare_dram_parameter("xin", [128, NP * 128], _f16,
                                      isOutput=False)
    wt_d = nc.declare_dram_parameter("wt", [128, F_], _f16, isOutput=False)
    g_d = nc.declare_dram_parameter("g", [128, NT * 32], _f16, isOutput=False)
    out_d = nc.declare_dram_parameter("out", [NMC, F_], _f16, isOutput=True)

    relu_fn = mybir.ActivationFunctionType.Relu
    max_op = mybir.AluOpType.max

    with TileContext(nc) as tc:
        with tc.tile_pool(name="const", bufs=1) as cpool, \
             tc.tile_pool(name="rbuf", bufs=6) as rpool, \
             tc.tile_pool(name="stage", bufs=1) as spool, \
             tc.tile_pool(name="pconv", bufs=3, space="PSUM") as pconv, \
             tc.tile_pool(name="psum", bufs=2, space="PSUM") as psump:
            xin_sb = cpool.tile([128, NP * 128], _f16)
            wt_sb = cpool.tile([128, F_], _f16)
            g_sb = cpool.tile([128, NT * 32], _f16)
            stage = spool.tile([NMC, F_], _f16)
            nc.sync.dma_start(out=xin_sb[:], in_=xin_d[:])
            nc.sync.dma_start(out=g_sb[:], in_=g_d[:])
            for i in range(4):
                s = slice(i * (F_ // 4), (i + 1) * (F_ // 4))
                nc.sync.dma_start(out=wt_sb[:, s], in_=wt_d[:, s])

            for c in range(NCH):
                fs = slice(c * FCH, (c + 1) * FCH)
                ps = psump.tile([128, 512], _f32, tag="ps")
                nvisit = [0, 0, 0, 0]
                pend = []       # (pair q, its relu'd r tile)
                for i, q in enumerate(PAIR_ORDER):
                    pc = pconv.tile([128, 1024], _f32, tag="pc")
                    nc.tensor.matmul(
                        out=pc[:, 0:FCH],
                        lhsT=xin_sb[0:KC, q * 128:(q + 1) * 128],
                        rhs=wt_sb[0:KC, fs], start=True, stop=True)
                    nc.tensor.matmul(
                        out=pc[:, 512:512 + FCH],
                        lhsT=xin_sb[64:64 + KC, q * 128:(q + 1) * 128],
                        rhs=wt_sb[64:64 + KC, fs], start=True, stop=True)
                    r = rpool.tile([128, 1024], _f16, tag="r")
                    if i in DVE_POS:
                        nc.vector.tensor_scalar(out=r[:, 0:1012],
                                                in0=pc[:, 0:1012],
                                                scalar1=0.0, scalar2=None,
                                                op0=max_op)
                    else:
                        nc.scalar.activation(out=r[:, 0:1012],
                                             in_=pc[:, 0:1012], func=relu_fn)
                    pend.append((q, r))
                    if i >= 1:
                        _emit_sums(nc, g_sb, ps, pend.pop(0), nvisit)
                _emit_sums(nc, g_sb, ps, pend.pop(0), nvisit)
                nc.vector.tensor_copy(out=stage[:, fs], in_=ps[:, 0:FCH])
                if c % 2 == 1:
                    osl = slice((c - 1) * FCH, (c + 1) * FCH)
                    nc.sync.dma_start(out=out_d[:, osl], in_=stage[:, osl])

    nc.compile()
    return nc


def _emit_sums(nc, g_sb, ps, qr, nvisit):
    q, r = qr
    for t, rs in ((2 * q, slice(0, FCH)), (2 * q + 1, slice(512, 512 + FCH))):
        grp = t // TPG
        nc.tensor.matmul(
            out=ps[32 * grp:32 * grp + 32, 0:FCH],
            lhsT=g_sb[:, 32 * t:32 * t + 32],
            rhs=r[:, rs],
            start=(nvisit[grp] == 0),
            stop=(nvisit[grp] == TPG - 1),
            skip_group_check=True,
            tile_position=(0, 32 * grp))
        nvisit[grp] += 1


def _get_program():
    global _cached_nc
    if _cached_nc is None:
        _cached_nc = _build_program()
    return _cached_nc


def _host_prep(seqs, weight, bias):
    s = np.asarray(seqs, np.float32).reshape(NM, L_, B_)[:, :, :A_]
    sw = sliding_window_view(s, K_, axis=1)          # (NM, 28, 20, 3)
    X = sw.transpose(3, 2, 0, 1).reshape(A_ * K_, NM, LOUT)
    X = np.concatenate([X, np.ones((1, NM, LOUT), np.float32)], axis=0)

    Wt = np.asarray(weight, np.float32).transpose(2, 1, 0).reshape(A_ * K_, F_)
    Wb = np.concatenate([Wt, np.asarray(bias, np.float32)[None, :]], axis=0)
    wt = np.zeros((128, F_), np.float32)
    wt[0:KC] = Wb
    wt[64:64 + KC] = Wb
    wt_f16 = wt.astype(np.float16)

    # G_t[j, m] = 1 iff position 128t+j belongs to output row m of col group t//7
    G = np.zeros((128, NT * 32), np.float16)
    for t in range(NT):
        nm_of_j = (128 * t + np.arange(128)) // LOUT
        G[np.arange(128), 32 * t + nm_of_j % 32] = 1.0

    in_maps = []
    for c in range(CORES):
        Xc = X[:, c * NMC:(c + 1) * NMC, :].reshape(KC, NT, 128)
        xin = np.zeros((128, NP, 128), np.float32)
        xin[0:KC] = Xc[:, 0::2]
        xin[64:64 + KC] = Xc[:, 1::2]
        in_maps.append({
            "xin": np.ascontiguousarray(
                xin.reshape(128, NP * 128)).astype(np.float16),
            "wt": wt_f16,
            "g": G,
        })
    return in_maps


def run_bass(seqs, weight, bias, trace=False):
    """Returns (out (32,32,8000) float32, exec_time_ns or None)."""
    nc = _get_program()
    in_maps = _host_prep(seqs, weight, bias)
    res = run_bass_kernel_spmd(nc, in_maps, list(range(CORES)), trace=trace)
    out = np.concatenate([res.results[c]["out"] for c in range(CORES)], axis=0)
    return out.reshape(N_, M_, F_).astype(np.float32), res.exec_time_ns


def kernel(seqs, weight, bias):
    out, _ = run_bass(seqs, weight, bias, trace=False)
    return out


# revision 5
# speedup vs baseline: 1.5342x; 1.3056x over previous
"""Trainium2 Bass kernel for nn_KmerEmbed: conv1d(one-hot kmer filters) + relu + window-sum.

Computes, for seqs (32,32,30,21), weight (8000,20,3), bias (8000,):
  out[n,m,f] = sum_l relu( sum_{a,j} seqs[n,m,l+j,a(<20)]*weight[f,a,j] + bias[f] )
with l over the 28 valid conv positions; returns (32,32,8000) float32.

Strategy (8 NeuronCores, data-parallel over the 1024 flattened rows, 128 rows/core):
  - im2col on host: the 128 rows x 28 positions = 3584 (nm,l) pairs are packed
    densely into 28 tiles of 128 PSUM partitions (full PE width), K=61 rows
    (60 one-hot taps + bias row).
  - conv = matmul vs the replicated filter matrix Wb (61,8000) f16; tiles are
    packed in pairs into PE row-groups (partitions 0-60 / 64-124) so two
    matmuls stream concurrently; f chunked by 500 (one PSUM bank per matmul),
    e/o halves of a 2-bank pair tile.
  - relu(conv) from PSUM by ScalarE (activation Relu) and VectorE (tensor_scalar
    max) in parallel, written to SBUF as float16, one instr per pair (FD=1012).
  - window-sum via matmul with 0/1 selection matrices G (128,32) f16, K=128:
    tile t feeds output column-group t//7 (4 col-groups run concurrently);
    7 tiles accumulate per group into one (128,500) PSUM bank per chunk.
    Pairs are visited in an order that rotates across the 4 col groups.
  - PSUM->SBUF copy converts to f16; DMA f16 to DRAM; host casts to f32.
"""

import os
import sys

import numpy as np
from numpy.lib.stride_tricks import sliding_window_view

for _p in ("/opt/trn_rl_repo", "/root/.axon_site/_ro/trn_rl_repo"):
    if os.path.isdir(_p) and _p not in sys.path:
        sys.path.insert(0, _p)

import concourse.bacc as bacc
import concourse.mybir as mybir
from concourse.tile import TileContext
from concourse.bass_utils import run_bass_kernel_spmd

# problem sizes (hardcoded per spec)
N_, M_, L_, B_ = 32, 32, 30, 21
A_, K_ = 20, 3
F_ = 8000
NM = N_ * M_              # 1024
CORES = 8
NMC = NM // CORES         # 128 rows per core
LOUT = L_ - K_ + 1        # 28
FLAT = NMC * LOUT         # 3584 (nm,l) positions per core
NT = FLAT // 128          # 28 tiles of 128 positions
NP = NT // 2              # 14 e/o pairs
TPG = NT // 4             # 7 tiles accumulate per output col group
KC = A_ * K_ + 1          # 61 = 60 + bias row
FCH = 500                 # matmul free-dim chunk (one PSUM bank)
NCH = F_ // FCH           # 16 chunks

_f32 = mybir.dt.float32
_f16 = mybir.dt.float16

# relu engine per pair position (6 on vector, 8 on scalar)
DVE_POS = frozenset((0, 3, 5, 7, 9, 12))

# pair bursts: convs for burst b and window-sums for burst b-1 are emitted as
# separate groups so LDWEIGHTS can pull ahead within each group (conv LDWs
# alternate PE row-groups; sum LDWs touch disjoint col-groups). Each burst
# spans 3-4 distinct sum col-groups to keep the 4-way sum concurrency.
PAIR_BURSTS = [(0, 4, 7), (11, 1, 5), (8, 12, 2), (6, 9, 13), (3, 10)]

_cached_nc = None


def _build_program():
    nc = bacc.Bacc("TRN2", target_bir_lowering=False, debug=False,
                   num_devices=CORES)
    xin_d = nc.declare_dram_parameter("xin", [128, NP * 128], _f16,
                                      isOutput=False)
    wt_d = nc.declare_dram_parameter("wt", [128, F_], _f16, isOutput=False)
    g_d = nc.declare_dram_parameter("g", [128, NT * 32], _f16, isOutput=False)
    out_d = nc.declare_dram_parameter("out", [NMC, F_], _f16, isOutput=True)

    relu_fn = mybir.ActivationFunctionType.Relu
    max_op = mybir.AluOpType.max

    with TileContext(nc) as tc:
        with tc.tile_pool(name="const", bufs=1) as cpool, \
             tc.tile_pool(name="rbuf", bufs=8) as rpool, \
             tc.tile_pool(name="stage", bufs=1) as spool, \
             tc.tile_pool(name="pconv", bufs=3, space="PSUM") as pconv, \
             tc.tile_pool(name="psum", bufs=2, space="PSUM") as psump:
            xin_sb = cpool.tile([128, NP * 128], _f16)
            wt_sb = cpool.tile([128, F_], _f16)
            g_sb = cpool.tile([128, NT * 32], _f16)
            stage = spool.tile([NMC, F_], _f16)
            nc.sync.dma_start(out=xin_sb[:], in_=xin_d[:])
            nc.sync.dma_start(out=g_sb[:], in_=g_d[:])
            for i in range(4):
                s = slice(i * (F_ // 4), (i + 1) * (F_ // 4))
                nc.sync.dma_start(out=wt_sb[:, s], in_=wt_d[:, s])

            for c in range(NCH):
                fs = slice(c * FCH, (c + 1) * FCH)
                ps = psump.tile([128, 512], _f32, tag="ps")
                nvisit = [0, 0, 0, 0]
                pend = []       # (pair q, its relu'd r tile)
                i = 0
                for b in range(len(PAIR_BURSTS) + 1):
                    if b < len(PAIR_BURSTS):
                        for q in PAIR_BURSTS[b]:
                            pc = pconv.tile([128, 1024], _f32, tag="pc")
                            nc.tensor.matmul(
                                out=pc[:, 0:FCH],
                                lhsT=xin_sb[0:KC, q * 128:(q + 1) * 128],
                                rhs=wt_sb[0:KC, fs], start=True, stop=True)
                            nc.tensor.matmul(
                                out=pc[:, 512:512 + FCH],
                                lhsT=xin_sb[64:64 + KC, q * 128:(q + 1) * 128],
                                rhs=wt_sb[64:64 + KC, fs],
                                start=True, stop=True)
                            r = rpool.tile([128, 1024], _f16, tag="r")
                            if i in DVE_POS:
                                nc.vector.tensor_scalar(out=r[:, 0:1012],
                                                        in0=pc[:, 0:1012],
                                                        scalar1=0.0,
                                                        scalar2=None,
                                                        op0=max_op)
                            else:
                                nc.scalar.activation(out=r[:, 0:1012],
                                                     in_=pc[:, 0:1012],
                                                     func=relu_fn)
                            pend.append((q, r))
                            i += 1
                    if b >= 1:
                        for qr in pend[:len(PAIR_BURSTS[b - 1])]:
                            _emit_sums(nc, g_sb, ps, qr, nvisit)
                        del pend[:len(PAIR_BURSTS[b - 1])]
                nc.vector.tensor_copy(out=stage[:, fs], in_=ps[:, 0:FCH])
                if c % 2 == 1:
                    osl = slice((c - 1) * FCH, (c + 1) * FCH)
                    nc.sync.dma_start(out=out_d[:, osl], in_=stage[:, osl])

    nc.compile()
    return nc


def _emit_sums(nc, g_sb, ps, qr, nvisit):
    q, r = qr
    for t, rs in ((2 * q, slice(0, FCH)), (2 * q + 1, slice(512, 512 + FCH))):
        grp = t // TPG
        nc.tensor.matmul(
            out=ps[32 * grp:32 * grp + 32, 0:FCH],
            lhsT=g_sb[:, 32 * t:32 * t + 32],
            rhs=r[:, rs],
            start=(nvisit[grp] == 0),
            stop=(nvisit[grp] == TPG - 1),
            skip_group_check=True,
            tile_position=(0, 32 * grp))
        nvisit[grp] += 1


def _get_program():
    global _cached_nc
    if _cached_nc is None:
        _cached_nc = _build_program()
    return _cached_nc


def _host_prep(seqs, weight, bias):
    s = np.asarray(seqs, np.float32).reshape(NM, L_, B_)[:, :, :A_]
    sw = sliding_window_view(s, K_, axis=1)          # (NM, 28, 20, 3)
    X = sw.transpose(3, 2, 0, 1).reshape(A_ * K_, NM, LOUT)
    X = np.concatenate([X, np.ones((1, NM, LOUT), np.float32)], axis=0)

    Wt = np.asarray(weight, np.float32).transpose(2, 1, 0).reshape(A_ * K_, F_)
    Wb = np.concatenate([Wt, np.asarray(bias, np.float32)[None, :]], axis=0)
    wt = np.zeros((128, F_), np.float32)
    wt[0:KC] = Wb
    wt[64:64 + KC] = Wb
    wt_f16 = wt.astype(np.float16)

    # G_t[j, m] = 1 iff position 128t+j belongs to output row m of col group t//7
    G = np.zeros((128, NT * 32), np.float16)
    for t in range(NT):
        nm_of_j = (128 * t + np.arange(128)) // LOUT
        G[np.arange(128), 32 * t + nm_of_j % 32] = 1.0

    in_maps = []
    for c in range(CORES):
        Xc = X[:, c * NMC:(c + 1) * NMC, :].reshape(KC, NT, 128)
        xin = np.zeros((128, NP, 128), np.float32)
        xin[0:KC] = Xc[:, 0::2]
        xin[64:64 + KC] = Xc[:, 1::2]
        in_maps.append({
            "xin": np.ascontiguousarray(
                xin.reshape(128, NP * 128)).astype(np.float16),
            "wt": wt_f16,
            "g": G,
        })
    return in_maps


def run_bass(seqs, weight, bias, trace=False):
    """Returns (out (32,32,8000) float32, exec_time_ns or None)."""
    nc = _get_program()
    in_maps = _host_prep(seqs, weight, bias)
    res = run_bass_kernel_spmd(nc, in_maps, list(range(CORES)), trace=trace)
    out = np.concatenate([res.results[c]["out"] for c in range(CORES)], axis=0)
    return out.reshape(N_, M_, F_).astype(np.float32), res.exec_time_ns


def kernel(seqs, weight, bias):
    out, _ = run_bass(seqs, weight, bias, trace=False)
    return out


# revision 7
# speedup vs baseline: 1.5612x; 1.0176x over previous
"""Trainium2 Bass kernel for nn_KmerEmbed: conv1d(one-hot kmer filters) + relu + window-sum.

Computes, for seqs (32,32,30,21), weight (8000,20,3), bias (8000,):
  out[n,m,f] = sum_l relu( sum_{a,j} seqs[n,m,l+j,a(<20)]*weight[f,a,j] + bias[f] )
with l over the 28 valid conv positions; returns (32,32,8000) float32.

Strategy (8 NeuronCores, data-parallel over the 1024 flattened rows, 128 rows/core):
  - im2col on host: the 128 rows x 28 positions = 3584 (nm,l) pairs are packed
    densely into 28 tiles of 128 PSUM partitions (full PE width), K=61 rows
    (60 one-hot taps + bias row).
  - conv = matmul vs the replicated filter matrix Wb (61,8000) f16; tiles are
    packed in pairs into PE row-groups (partitions 0-60 / 64-124) so two
    matmuls stream concurrently; f chunked by 500 (one PSUM bank per matmul),
    e/o halves of a 2-bank pair tile.
  - relu(conv) from PSUM by ScalarE (activation Relu) and VectorE (tensor_scalar
    max) in parallel, written to SBUF as float16, one instr per pair (FD=1012).
  - window-sum via matmul with 0/1 selection matrices G (128,32) f16, K=128:
    tile t feeds output column-group t//7 (4 col-groups run concurrently);
    7 tiles accumulate per group into one (128,500) PSUM bank per chunk.
    Pairs are visited in an order that rotates across the 4 col groups.
  - PSUM->SBUF copy converts to f16; DMA f16 to DRAM; host casts to f32.
"""

import os
import sys

import numpy as np
from numpy.lib.stride_tricks import sliding_window_view

for _p in ("/opt/trn_rl_repo", "/root/.axon_site/_ro/trn_rl_repo"):
    if os.path.isdir(_p) and _p not in sys.path:
        sys.path.insert(0, _p)

import concourse.bacc as bacc
import concourse.mybir as mybir
from concourse.tile import TileContext
from concourse.bass_utils import run_bass_kernel_spmd

# problem sizes (hardcoded per spec)
N_, M_, L_, B_ = 32, 32, 30, 21
A_, K_ = 20, 3
F_ = 8000
NM = N_ * M_              # 1024
CORES = 8
NMC = NM // CORES         # 128 rows per core
LOUT = L_ - K_ + 1        # 28
FLAT = NMC * LOUT         # 3584 (nm,l) positions per core
NT = FLAT // 128          # 28 tiles of 128 positions
NP = NT // 2              # 14 e/o pairs
TPG = NT // 4             # 7 tiles accumulate per output col group
KC = A_ * K_ + 1          # 61 = 60 + bias row
FCH = 500                 # matmul free-dim chunk (one PSUM bank)
NCH = F_ // FCH           # 16 chunks

_f32 = mybir.dt.float32
_f16 = mybir.dt.float16

# relu engine per pair position (7 on vector, 7 + final cast on scalar)
DVE_POS = frozenset((0, 2, 4, 6, 8, 10, 12))

# pair bursts: convs for burst b and window-sums for burst b-1 are emitted as
# separate groups so LDWEIGHTS can pull ahead within each group (conv LDWs
# alternate PE row-groups; sum LDWs touch disjoint col-groups). Each burst
# spans 3-4 distinct sum col-groups to keep the 4-way sum concurrency.
PAIR_BURSTS = [(0, 4, 7), (11, 1, 5), (8, 12, 2), (6, 9, 13), (3, 10)]

_cached_nc = None


def _build_program():
    nc = bacc.Bacc("TRN2", target_bir_lowering=False, debug=False,
                   num_devices=CORES)
    xin_d = nc.declare_dram_parameter("xin", [128, NP * 128], _f16,
                                      isOutput=False)
    wt_d = nc.declare_dram_parameter("wt", [128, F_], _f16, isOutput=False)
    g_d = nc.declare_dram_parameter("g", [128, NT * 32], _f16, isOutput=False)
    out_d = nc.declare_dram_parameter("out", [NMC, F_], _f16, isOutput=True)

    relu_fn = mybir.ActivationFunctionType.Relu
    max_op = mybir.AluOpType.max

    with TileContext(nc) as tc:
        with tc.tile_pool(name="const", bufs=1) as cpool, \
             tc.tile_pool(name="rbuf", bufs=8) as rpool, \
             tc.tile_pool(name="stage", bufs=1) as spool, \
             tc.tile_pool(name="pconv", bufs=3, space="PSUM") as pconv, \
             tc.tile_pool(name="psum", bufs=2, space="PSUM") as psump:
            xin_sb = cpool.tile([128, NP * 128], _f16)
            wt_sb = cpool.tile([128, F_], _f16)
            g_sb = cpool.tile([128, NT * 32], _f16)
            stage = spool.tile([NMC, F_], _f16)
            nc.sync.dma_start(out=xin_sb[:], in_=xin_d[:])
            nc.sync.dma_start(out=g_sb[:], in_=g_d[:])
            for i in range(4):
                s = slice(i * (F_ // 4), (i + 1) * (F_ // 4))
                nc.sync.dma_start(out=wt_sb[:, s], in_=wt_d[:, s])

            for c in range(NCH):
                fs = slice(c * FCH, (c + 1) * FCH)
                ps = psump.tile([128, 512], _f32, tag="ps")
                nvisit = [0, 0, 0, 0]
                pend = []       # (pair q, its relu'd r tile)
                i = 0
                for b in range(len(PAIR_BURSTS) + 1):
                    if b < len(PAIR_BURSTS):
                        for q in PAIR_BURSTS[b]:
                            pc = pconv.tile([128, 1024], _f32, tag="pc")
                            nc.tensor.matmul(
                                out=pc[:, 0:FCH],
                                lhsT=xin_sb[0:KC, q * 128:(q + 1) * 128],
                                rhs=wt_sb[0:KC, fs], start=True, stop=True)
                            nc.tensor.matmul(
                                out=pc[:, 512:512 + FCH],
                                lhsT=xin_sb[64:64 + KC, q * 128:(q + 1) * 128],
                                rhs=wt_sb[64:64 + KC, fs],
                                start=True, stop=True)
                            r = rpool.tile([128, 1024], _f16, tag="r")
                            if i in DVE_POS:
                                nc.vector.tensor_scalar(out=r[:, 0:1012],
                                                        in0=pc[:, 0:1012],
                                                        scalar1=0.0,
                                                        scalar2=None,
                                                        op0=max_op)
                            else:
                                nc.scalar.activation(out=r[:, 0:1012],
                                                     in_=pc[:, 0:1012],
                                                     func=relu_fn)
                            pend.append((q, r))
                            i += 1
                    # window-sums lag the convs by TWO bursts so conv dispatch
                    # never queues behind relu-gated sum matmuls in the PE FIFO
                    if b >= 2:
                        for qr in pend[:len(PAIR_BURSTS[b - 2])]:
                            _emit_sums(nc, g_sb, ps, qr, nvisit)
                        del pend[:len(PAIR_BURSTS[b - 2])]
                for qr in pend:
                    _emit_sums(nc, g_sb, ps, qr, nvisit)
                pend.clear()
                nc.scalar.copy(out=stage[:, fs], in_=ps[:, 0:FCH])
                if c % 2 == 1:
                    osl = slice((c - 1) * FCH, (c + 1) * FCH)
                    nc.sync.dma_start(out=out_d[:, osl], in_=stage[:, osl])

    nc.compile()
    return nc


def _emit_sums(nc, g_sb, ps, qr, nvisit):
    q, r = qr
    for t, rs in ((2 * q, slice(0, FCH)), (2 * q + 1, slice(512, 512 + FCH))):
        grp = t // TPG
        nc.tensor.matmul(
            out=ps[32 * grp:32 * grp + 32, 0:FCH],
            lhsT=g_sb[:, 32 * t:32 * t + 32],
            rhs=r[:, rs],
            start=(nvisit[grp] == 0),
            stop=(nvisit[grp] == TPG - 1),
            skip_group_check=True,
            tile_position=(0, 32 * grp))
        nvisit[grp] += 1


def _get_program():
    global _cached_nc
    if _cached_nc is None:
        _cached_nc = _build_program()
    return _cached_nc


def _host_prep(seqs, weight, bias):
    s = np.asarray(seqs, np.float32).reshape(NM, L_, B_)[:, :, :A_]
    sw = sliding_window_view(s, K_, axis=1)          # (NM, 28, 20, 3)
    X = sw.transpose(3, 2, 0, 1).reshape(A_ * K_, NM, LOUT)
    X = np.concatenate([X, np.ones((1, NM, LOUT), np.float32)], axis=0)

    Wt = np.asarray(weight, np.float32).transpose(2, 1, 0).reshape(A_ * K_, F_)
    Wb = np.concatenate([Wt, np.asarray(bias, np.float32)[None, :]], axis=0)
    wt = np.zeros((128, F_), np.float32)
    wt[0:KC] = Wb
    wt[64:64 + KC] = Wb
    wt_f16 = wt.astype(np.float16)

    # G_t[j, m] = 1 iff position 128t+j belongs to output row m of col group t//7
    G = np.zeros((128, NT * 32), np.float16)
    for t in range(NT):
        nm_of_j = (128 * t + np.arange(128)) // LOUT
        G[np.arange(128), 32 * t + nm_of_j % 32] = 1.0

    in_maps = []
    for c in range(CORES):
        Xc = X[:, c * NMC:(c + 1) * NMC, :].reshape(KC, NT, 128)
        xin = np.zeros((128, NP, 128), np.float32)
        xin[0:KC] = Xc[:, 0::2]
        xin[64:64 + KC] = Xc[:, 1::2]
        in_maps.append({
            "xin": np.ascontiguousarray(
                xin.reshape(128, NP * 128)).astype(np.float16),
            "wt": wt_f16,
            "g": G,
        })
    return in_maps


def run_bass(seqs, weight, bias, trace=False):
    """Returns (out (32,32,8000) float32, exec_time_ns or None)."""
    nc = _get_program()
    in_maps = _host_prep(seqs, weight, bias)
    res = run_bass_kernel_spmd(nc, in_maps, list(range(CORES)), trace=trace)
    out = np.concatenate([res.results[c]["out"] for c in range(CORES)], axis=0)
    return out.reshape(N_, M_, F_).astype(np.float32), res.exec_time_ns


def kernel(seqs, weight, bias):
    out, _ = run_bass(seqs, weight, bias, trace=False)
    return out


# revision 10
# speedup vs baseline: 1.7492x; 1.1205x over previous
"""Trainium2 Bass kernel for nn_KmerEmbed: conv1d(one-hot kmer filters) + relu + window-sum.

Computes, for seqs (32,32,30,21), weight (8000,20,3), bias (8000,):
  out[n,m,f] = sum_l relu( sum_{a,j} seqs[n,m,l+j,a(<20)]*weight[f,a,j] + bias[f] )
with l over the 28 valid conv positions; returns (32,32,8000) float32.

Strategy (8 NeuronCores, data-parallel over the 1024 flattened rows, 128 rows/core):
  - im2col on host: the 128 rows x 28 positions = 3584 (nm,l) pairs are packed
    densely into 28 tiles of 128 PSUM partitions (full PE width), K=61 rows
    (60 one-hot taps + bias row).
  - conv = matmul vs the replicated filter matrix Wb (61,8000) f16; tiles are
    packed in pairs into PE row-groups (partitions 0-60 / 64-124) so two
    matmuls stream concurrently; f chunked by 512 (one PSUM bank per matmul),
    e/o halves of a 2-bank pair tile.
  - relu(conv) from PSUM by ScalarE (activation Relu) and VectorE (tensor_scalar
    max) in parallel, written to SBUF as float16, one instr per pair (FD=1024).
  - window-sum via matmul with 0/1 selection matrices G (128,32) f16, K=128:
    tile t feeds output column-group t//7 (4 col-groups run concurrently);
    7 tiles accumulate per group into one (128,512) PSUM bank per chunk.
  - a flat software pipeline over (chunk, burst) units: conv bursts of 3 pairs,
    window-sum bursts lag by 2 units, PSUM->SBUF casts lag 1 more unit. This
    keeps LDWEIGHTS hidden inside homogeneous bursts (conv LDWs alternate PE
    row-groups, sum LDWs touch disjoint col-groups) and removes chunk-boundary
    stalls on every engine.
  - output staged as f16, DMA'd per 2 chunks; host casts to f32.
"""

import os
import sys

import numpy as np
from numpy.lib.stride_tricks import sliding_window_view

for _p in ("/opt/trn_rl_repo", "/root/.axon_site/_ro/trn_rl_repo"):
    if os.path.isdir(_p) and _p not in sys.path:
        sys.path.insert(0, _p)

import concourse.bacc as bacc
import concourse.mybir as mybir
from concourse.tile import TileContext
from concourse.bass_utils import run_bass_kernel_spmd

# problem sizes (hardcoded per spec)
N_, M_, L_, B_ = 32, 32, 30, 21
A_, K_ = 20, 3
F_ = 8000
NM = N_ * M_              # 1024
CORES = 8
NMC = NM // CORES         # 128 rows per core
LOUT = L_ - K_ + 1        # 28
FLAT = NMC * LOUT         # 3584 (nm,l) positions per core
NT = FLAT // 128          # 28 tiles of 128 positions
NP = NT // 2              # 14 e/o pairs
TPG = NT // 4             # 7 tiles accumulate per output col group
KC = A_ * K_ + 1          # 61 = 60 + bias row
FCH = 512                 # f chunk (one PSUM bank per conv matmul)
NCH = (F_ + FCH - 1) // FCH   # 16 chunks (last is 368 wide)

_f32 = mybir.dt.float32
_f16 = mybir.dt.float16

# pair bursts: convs for burst u and window-sums for burst u-2 are emitted as
# separate groups so LDWEIGHTS can pull ahead within each group. Each burst
# spans 3-4 distinct sum col-groups to keep the 4-way sum concurrency.
PAIR_BURSTS = [(0, 4, 7), (11, 1, 5), (8, 12, 2), (6, 9, 13), (3, 10)]
NB = len(PAIR_BURSTS)

_cached_nc = None


def _build_program():
    nc = bacc.Bacc("TRN2", target_bir_lowering=False, debug=False,
                   num_devices=CORES)
    xin_d = nc.declare_dram_parameter("xin", [128, NP * 128], _f16,
                                      isOutput=False)
    wt_d = nc.declare_dram_parameter("wt", [128, F_], _f16, isOutput=False)
    g_d = nc.declare_dram_parameter("g", [128, NT * 32], _f16, isOutput=False)
    out_d = nc.declare_dram_parameter("out", [NMC, F_], _f16, isOutput=True)

    relu_fn = mybir.ActivationFunctionType.Relu
    max_op = mybir.AluOpType.max

    with TileContext(nc) as tc:
        with tc.tile_pool(name="const", bufs=1) as cpool, \
             tc.tile_pool(name="rbuf", bufs=12) as rpool, \
             tc.tile_pool(name="stage", bufs=1) as spool, \
             tc.tile_pool(name="pconv", bufs=3, space="PSUM") as pconv, \
             tc.tile_pool(name="psum", bufs=2, space="PSUM") as psump:
            xin_sb = cpool.tile([128, NP * 128], _f16)
            wt_sb = cpool.tile([128, F_], _f16)
            g_sb = cpool.tile([128, NT * 32], _f16)
            stage = spool.tile([NMC, F_], _f16)
            # preload order: first conv chunk is runnable ~2us in
            nc.sync.dma_start(out=xin_sb[:], in_=xin_d[:])
            nc.sync.dma_start(out=wt_sb[:, 0:FCH], in_=wt_d[:, 0:FCH])
            nc.sync.dma_start(out=g_sb[:], in_=g_d[:])
            for c in range(1, NCH):
                s = slice(c * FCH, min(F_, (c + 1) * FCH))
                nc.sync.dma_start(out=wt_sb[:, s], in_=wt_d[:, s])

            units = [(c, b) for c in range(NCH) for b in range(NB)]
            pend = []        # per unit: (chunk, [(tile pair, r tile), ...])
            ps_of = {}       # chunk -> accumulating sum tile
            nvisit = {}      # chunk -> per-col-group visit counts
            casts = []       # chunks whose sums are done, cast not yet emitted
            gi = 0           # global pair index (relu engine alternation)

            def do_sums(cu, qrs):
                if cu not in ps_of:
                    ps_of[cu] = psump.tile([128, 512], _f32, tag="ps",
                                           name=f"ps{cu}")
                    nvisit[cu] = [0, 0, 0, 0]
                w = min(F_, (cu + 1) * FCH) - cu * FCH
                for q, r in qrs:
                    for t, rs in ((2 * q, slice(0, w)),
                                  (2 * q + 1, slice(512, 512 + w))):
                        grp = t // TPG
                        nc.tensor.matmul(
                            out=ps_of[cu][32 * grp:32 * grp + 32, 0:w],
                            lhsT=g_sb[:, 32 * t:32 * t + 32],
                            rhs=r[:, rs],
                            start=(nvisit[cu][grp] == 0),
                            stop=(nvisit[cu][grp] == TPG - 1),
                            skip_group_check=True,
                            tile_position=(0, 32 * grp))
                        nvisit[cu][grp] += 1

            def do_cast(cu):
                fs = slice(cu * FCH, min(F_, (cu + 1) * FCH))
                w = fs.stop - fs.start
                nc.scalar.copy(out=stage[:, fs], in_=ps_of.pop(cu)[:, 0:w])
                del nvisit[cu]
                if cu % 2 == 1:
                    osl = slice((cu - 1) * FCH, min(F_, (cu + 1) * FCH))
                    nc.sync.dma_start(out=out_d[:, osl], in_=stage[:, osl])

            for u, (c, b) in enumerate(units):
                fs = slice(c * FCH, min(F_, (c + 1) * FCH))
                w = fs.stop - fs.start
                qrs = []
                for q in PAIR_BURSTS[b]:
                    pc = pconv.tile([128, 1024], _f32, tag="pc")
                    nc.tensor.matmul(
                        out=pc[:, 0:w],
                        lhsT=xin_sb[0:KC, q * 128:(q + 1) * 128],
                        rhs=wt_sb[0:KC, fs], start=True, stop=True)
                    nc.tensor.matmul(
                        out=pc[:, 512:512 + w],
                        lhsT=xin_sb[64:64 + KC, q * 128:(q + 1) * 128],
                        rhs=wt_sb[64:64 + KC, fs], start=True, stop=True)
                    r = rpool.tile([128, 1024], _f16, tag="r")
                    if gi % 2 == 0:
                        nc.vector.tensor_scalar(out=r[:, 0:512 + w],
                                                in0=pc[:, 0:512 + w],
                                                scalar1=0.0, scalar2=None,
                                                op0=max_op)
                    else:
                        nc.scalar.activation(out=r[:, 0:512 + w],
                                             in_=pc[:, 0:512 + w],
                                             func=relu_fn)
                    gi += 1
                    qrs.append((q, r))
                pend.append((c, qrs))
                if casts:
                    do_cast(casts.pop(0))
                if u >= 2:
                    cu, qrs_u = pend.pop(0)
                    do_sums(cu, qrs_u)
                    if (u - 2) % NB == NB - 1:
                        casts.append(cu)
            for cu, qrs_u in pend:
                do_sums(cu, qrs_u)
                if casts:
                    do_cast(casts.pop(0))
            casts.append(NCH - 1)
            while casts:
                do_cast(casts.pop(0))

    nc.compile()
    return nc


def _get_program():
    global _cached_nc
    if _cached_nc is None:
        _cached_nc = _build_program()
    return _cached_nc


def _host_prep(seqs, weight, bias):
    s = np.asarray(seqs, np.float32).reshape(NM, L_, B_)[:, :, :A_]
    sw = sliding_window_view(s, K_, axis=1)          # (NM, 28, 20, 3)
    X = sw.transpose(3, 2, 0, 1).reshape(A_ * K_, NM, LOUT)
    X = np.concatenate([X, np.ones((1, NM, LOUT), np.float32)], axis=0)

    Wt = np.asarray(weight, np.float32).transpose(2, 1, 0).reshape(A_ * K_, F_)
    Wb = np.concatenate([Wt, np.asarray(bias, np.float32)[None, :]], axis=0)
    wt = np.zeros((128, F_), np.float32)
    wt[0:KC] = Wb
    wt[64:64 + KC] = Wb
    wt_f16 = wt.astype(np.float16)

    # G_t[j, m] = 1 iff position 128t+j belongs to output row m of col group t//7
    G = np.zeros((128, NT * 32), np.float16)
    for t in range(NT):
        nm_of_j = (128 * t + np.arange(128)) // LOUT
        G[np.arange(128), 32 * t + nm_of_j % 32] = 1.0

    in_maps = []
    for c in range(CORES):
        Xc = X[:, c * NMC:(c + 1) * NMC, :].reshape(KC, NT, 128)
        xin = np.zeros((128, NP, 128), np.float32)
        xin[0:KC] = Xc[:, 0::2]
        xin[64:64 + KC] = Xc[:, 1::2]
        in_maps.append({
            "xin": np.ascontiguousarray(
                xin.reshape(128, NP * 128)).astype(np.float16),
            "wt": wt_f16,
            "g": G,
        })
    return in_maps


def run_bass(seqs, weight, bias, trace=False):
    """Returns (out (32,32,8000) float32, exec_time_ns or None)."""
    nc = _get_program()
    in_maps = _host_prep(seqs, weight, bias)
    res = run_bass_kernel_spmd(nc, in_maps, list(range(CORES)), trace=trace)
    out = np.concatenate([res.results[c]["out"] for c in range(CORES)], axis=0)
    return out.reshape(N_, M_, F_).astype(np.float32), res.exec_time_ns


def kernel(seqs, weight, bias):
    out, _ = run_bass(seqs, weight, bias, trace=False)
    return out
